# revision 9
# baseline (speedup 1.0000x reference)
"""MultiHeadSelfAttention (L=2048, N=4, E=1024, H=16, causal, sum-reduced)
on 8 Trainium2 NeuronCores.

Math: the reference computes sum(attn_out @ wout.T). Because the final
reduction is a full sum, the output projection collapses to a dot with
wsum = wout.sum(0), and the V projection collapses into per-head vectors
u_h = wv_h.T @ wsum_h, giving per (batch n, head h):

    scalar += sum_l  [ sum_{m<=l} e(s_lm) * vw_m ] / [ sum_{m<=l} e(s_lm) ]

with s = (x wq_h.T)(x wk_h.T).T / sqrt(d),  e = exp (no row-max needed:
|s| stays far below f32 exp overflow),  vw = x @ u_h.

Sharding: 64 (n, h) units over 8 cores -> core c handles n = c//2 and 8
heads h0 = 8*(c%2). Per core everything stays on-chip: q/k projections
(bf16 matmuls into f32 PSUM), per-head 2048x2048 causal scores in
transposed orientation (m on partitions), exp on ScalarE (scale=1/8
folded in, bf16 out), and num/den via PE matmuls against
[vw_hi | vw_lo | 1] bf16 columns (hi/lo split keeps the vw path at
f32-class accuracy; vw itself is computed on-chip from hi/lo-split x).
The host divides num/den and sums (tiny, well-conditioned).
"""

import os

import ml_dtypes
import numpy as np

import concourse.bacc as bacc
import concourse.mybir as mybir
import concourse.tile as tile
from concourse.bass_utils import run_bass_kernel_spmd
from concourse.masks import make_identity

ml_bf16 = ml_dtypes.bfloat16

L = 2048
N = 4
E = 1024
H = 16
D = 64
HPC = 8  # heads per core
NCORES = 8
KS = E // 128  # 8 contraction subtiles
LB = 512  # l-block (matmul free dim)
NLB = L // LB  # 4
MB = 128  # m-block (scores partition dim)
NMO = L // MB  # 16
G = 3  # score blocks per PSUM tile / ACT instruction

F32 = mybir.dt.float32
BF16 = mybir.dt.bfloat16

_nc_cache = None


def _emit_main_head(nc, h, qt_sb, kt_sb, mask_sb, redw, spsum, rpsum, epool,
                    ndpool, numden):
    """Scores + exp + mask + num/den reduction for one local head.

    Diagonal blocks are width-trimmed (o=2 -> l in [256,512), o=3 -> l in
    [384,512)); reduction matmuls trail the score matmuls by RED_DELAY
    groups so the PE never stalls waiting for ScalarE's exp."""
    jg = h // 2
    po = (h % 2) * 64
    RED_DELAY = 2
    for lb in range(NLB):
        # entries: (mo, l_off, width, diag_o or None)
        entries = [(mo, 0, LB, None) for mo in range(4 * lb)]
        for o, (off, w) in enumerate(((0, 512), (0, 512), (256, 256), (384, 128))):
            entries.append((4 * lb + o, off, w, o))
        # groups: full blocks in triplets, the 4 diagonal blocks together
        full = entries[: 4 * lb]
        groups = [full[i : i + G] for i in range(0, len(full), G)]
        groups = [g for g in groups if g] + [entries[4 * lb :]]
        n_entries = len(entries)

        psr = rpsum.tile([3, LB], F32, tag="r", name="psr")
        pending = []

        def emit_red(grp_info, first_idx_of_lb=0):
            et, grp, offs = grp_info
            for (mo, off, w, o), poff in zip(grp, offs):
                ei = next(i for i, e in enumerate(entries) if e[0] == mo)
                nc.tensor.matmul(
                    psr[:, off : off + w],
                    redw[:, mo, h, :],
                    et[:, poff : poff + w],
                    start=(ei == 0),
                    stop=(ei == n_entries - 1),
                )

        for gi, grp in enumerate(groups):
            offs = []
            tot = 0
            for _, _, w, _ in grp:
                offs.append(tot)
                tot += w
            ps = spsum.tile([128, G * LB], F32, tag="s", name="ps_scores")
            for (mo, off, w, o), poff in zip(grp, offs):
                nc.tensor.matmul(
                    ps[:, poff : poff + w],
                    kt_sb[po : po + 64, jg, mo * MB : (mo + 1) * MB],
                    qt_sb[po : po + 64, jg, lb * LB + off : lb * LB + off + w],
                    start=True,
                    stop=True,
                    tile_position=(po, 0),
                )
            et = epool.tile([128, G * LB], BF16, tag="e", name="et")
            nc.scalar.activation(
                et[:, :tot], ps[:, :tot],
                mybir.ActivationFunctionType.Exp,
                scale=float(D) ** -0.5,
            )
            for (mo, off, w, o), poff in zip(grp, offs):
                if o is not None:
                    nc.vector.tensor_mul(
                        et[:, poff : poff + w],
                        et[:, poff : poff + w],
                        mask_sb[:, o, off : off + w],
                    )
            pending.append((et, grp, offs))
            if len(pending) > RED_DELAY:
                emit_red(pending.pop(0))
        for grp_info in pending:
            emit_red(grp_info)
        ndt = ndpool.tile([3, LB], F32, tag="nd", name="ndt")
        nc.vector.tensor_copy(ndt[:], psr[:])
        nc.sync.dma_start(numden[:, h, lb, :], ndt[:])


def _build():
    nc = bacc.Bacc(None, target_bir_lowering=False, name="mhsa_sum")

    xt = nc.dram_tensor("xt", [E, L], BF16, kind="ExternalInput")
    xtl = nc.dram_tensor("xtl", [E, L], BF16, kind="ExternalInput")
    wqt = nc.dram_tensor("wqt", [E, HPC * D], BF16, kind="ExternalInput")
    wkt = nc.dram_tensor("wkt", [E, HPC * D], BF16, kind="ExternalInput")
    u2 = nc.dram_tensor("u2", [E, 2, HPC], BF16, kind="ExternalInput")  # hi, lo
    mask = nc.dram_tensor("mask", [128, 4, LB], BF16, kind="ExternalInput")
    numden = nc.dram_tensor("numden", [3, HPC, NLB, LB], F32, kind="ExternalOutput")

    xt3 = xt.rearrange("(ko p) l -> p ko l", p=128)
    xtl3 = xtl.rearrange("(ko p) l -> p ko l", p=128)
    wqt3 = wqt.rearrange("(ko p) j -> p ko j", p=128)
    wkt3 = wkt.rearrange("(ko p) j -> p ko j", p=128)
    u23 = u2.rearrange("(ko p) t h -> p ko t h", p=128)

    with tile.TileContext(nc) as tc:
        with (
            tc.tile_pool(name="const", bufs=1) as const,
            tc.tile_pool(name="epool", bufs=6) as epool,
            tc.tile_pool(name="ndpool", bufs=2) as ndpool,
            tc.tile_pool(name="spsum", bufs=2, space="PSUM") as spsum,
            tc.tile_pool(name="rpsum", bufs=2, space="PSUM") as rpsum,
        ):
            wq_sb = const.tile([128, KS, HPC * D], BF16)
            wk_sb = const.tile([128, KS, HPC * D], BF16)
            u_sb = const.tile([128, KS, 2, HPC], BF16)
            mask_sb = const.tile([128, 4, LB], BF16)
            xt_sb = const.tile([128, KS, L], BF16)
            xtl_sb = const.tile([128, KS, L], BF16)
            qt_sb = const.tile([128, 4, L], BF16)
            kt_sb = const.tile([128, 4, L], BF16)
            redw = const.tile([128, NMO, HPC, 3], BF16)
            ident8 = const.tile([8, 8], F32)
            vwsb = const.tile([8, L], F32)

            nc.sync.dma_start(u_sb[:], u23[:])
            nc.sync.dma_start(wq_sb[:], wqt3[:])
            nc.sync.dma_start(wk_sb[:], wkt3[:])
            nc.sync.dma_start(mask_sb[:], mask[:])
            for ks in range(KS):
                nc.sync.dma_start(xt_sb[:, ks, :], xt3[:, ks, :])
            for ks in range(KS):
                nc.sync.dma_start(xtl_sb[:, ks, :], xtl3[:, ks, :])

            make_identity(nc, ident8[:])
            nc.vector.memset(redw[:, :, :, 2], 1.0)

            # vw[h, m] = sum_e x[m, e] u[e, h], natural orientation, three
            # bf16 hi/lo cross terms accumulated in f32 PSUM.
            for lbs in ((0, 1, 2), (3,)):
                psv = spsum.tile([8, len(lbs) * LB], F32, tag="s", name="psv")
                for li, lb in enumerate(lbs):
                    for ti, (xs, ut) in enumerate(
                        ((xt_sb, 0), (xt_sb, 1), (xtl_sb, 0))
                    ):
                        for ks in range(KS):
                            nc.tensor.matmul(
                                psv[:, li * LB : (li + 1) * LB],
                                u_sb[:, ks, ut, :],
                                xs[:, ks, lb * LB : (lb + 1) * LB],
                                start=(ti == 0 and ks == 0),
                                stop=(ti == 2 and ks == KS - 1),
                            )
                nc.vector.tensor_copy(
                    vwsb[:, lbs[0] * LB : (lbs[-1] + 1) * LB], psv[:]
                )

            # transpose vw to [m, h] per 128-m-block, then split-store
            # hi/lo bf16 into the reduction-weight columns.
            for mo in range(NMO):
                psT = rpsum.tile([128, 8], F32, tag="r", name="psT")
                nc.tensor.transpose(
                    psT[:], vwsb[:, mo * MB : (mo + 1) * MB], ident8[:]
                )
                nc.vector.tensor_copy(redw[:, mo, :, 0], psT[:])
                nc.vector.tensor_sub(redw[:, mo, :, 1], psT[:], redw[:, mo, :, 0])

            # q/k projections interleaved with per-head-pair main loops so
            # the PE never drains while ACT catches up.
            for jg in range(4):
                for w_sb, out_sb in ((wq_sb, qt_sb), (wk_sb, kt_sb)):
                    for lbs in ((0, 1, 2), (3,)):
                        ps = spsum.tile([128, G * LB], F32, tag="s", name="ps_proj")
                        for li, lb in enumerate(lbs):
                            for ks in range(KS):
                                nc.tensor.matmul(
                                    ps[:, li * LB : (li + 1) * LB],
                                    w_sb[:, ks, jg * 128 : (jg + 1) * 128],
                                    xt_sb[:, ks, lb * LB : (lb + 1) * LB],
                                    start=(ks == 0),
                                    stop=(ks == KS - 1),
                                )
                        nc.vector.tensor_copy(
                            out_sb[:, jg, lbs[0] * LB : (lbs[-1] + 1) * LB],
                            ps[:, : len(lbs) * LB],
                        )
                for h in (2 * jg, 2 * jg + 1):
                    _emit_main_head(nc, h, qt_sb, kt_sb, mask_sb, redw,
                                    spsum, rpsum, epool, ndpool, numden)

    nc.compile()
    return nc


def _split_bf16(a):
    hi = a.astype(np.float32).astype(ml_bf16)
    lo = (a.astype(np.float32) - hi.astype(np.float32)).astype(ml_bf16)
    return hi, lo


def _prep_core_inputs(x, wqkv, wout):
    """Host-side sharding + weight folding (data marshaling only)."""
    x = np.ascontiguousarray(np.asarray(x, dtype=np.float32))
    wqkv = np.asarray(wqkv, dtype=np.float32)
    wout = np.asarray(wout, dtype=np.float32)

    wsum = wout.astype(np.float64).sum(axis=0)  # [E]

    # causal mask for transposed-score diagonal blocks:
    # valid when f >= 128*o + p
    p = np.arange(128)[:, None, None]
    o = np.arange(4)[None, :, None]
    f = np.arange(LB)[None, None, :]
    mask_np = (f >= 128 * o + p).astype(ml_bf16)

    in_maps = []
    for c in range(NCORES):
        n = c // 2
        h0 = HPC * (c % 2)
        xt_f32 = np.ascontiguousarray(x[:, n, :].T)  # [E, L]
        xt_hi, xt_lo = _split_bf16(xt_f32)
        wq = wqkv[h0 * D : (h0 + HPC) * D, :]  # [512, E]
        wk = wqkv[E + h0 * D : E + (h0 + HPC) * D, :]
        wqt = np.ascontiguousarray(wq.T).astype(ml_bf16)
        wkt = np.ascontiguousarray(wk.T).astype(ml_bf16)
        u = np.empty((E, HPC), dtype=np.float32)
        for i in range(HPC):
            h = h0 + i
            wv_h = wqkv[2 * E + h * D : 2 * E + (h + 1) * D, :].astype(np.float64)
            u[:, i] = (wv_h.T @ wsum[h * D : (h + 1) * D]).astype(np.float32)
        u_hi, u_lo = _split_bf16(u)
        u2 = np.stack([u_hi, u_lo], axis=1)  # [E, 2, HPC]
        in_maps.append(
            {
                "xt": xt_hi,
                "xtl": xt_lo,
                "wqt": wqt,
                "wkt": wkt,
                "u2": np.ascontiguousarray(u2),
                "mask": mask_np,
            }
        )
    return in_maps


def kernel(x, wqkv, wout):
    global _nc_cache
    if _nc_cache is None:
        _nc_cache = _build()
    nc = _nc_cache

    in_maps = _prep_core_inputs(x, wqkv, wout)
    trace = os.environ.get("KERNEL_TRACE") == "1"
    res = run_bass_kernel_spmd(
        nc, in_maps, core_ids=list(range(NCORES)), trace=trace
    )
    if trace:
        kernel.last_result = res

    total = 0.0
    for c in range(NCORES):
        nd = res.results[c]["numden"].astype(np.float64)
        total += ((nd[0] + nd[1]) / nd[2]).sum()
    return np.array(total, dtype=np.float32)


# revision 16
# speedup vs baseline: 1.0335x; 1.0335x over previous
"""MultiHeadSelfAttention (L=2048, N=4, E=1024, H=16, causal, sum-reduced)
on 8 Trainium2 NeuronCores.

Math: the reference computes sum(attn_out @ wout.T). Because the final
reduction is a full sum, the output projection collapses to a dot with
wsum = wout.sum(0), and the V projection collapses into per-head vectors
u_h = wv_h.T @ wsum_h, giving per (batch n, head h):

    scalar += sum_l  [ sum_{m<=l} e(s_lm) * vw_m ] / [ sum_{m<=l} e(s_lm) ]

with s = (x wq_h.T)(x wk_h.T).T / sqrt(d),  e = exp (no row-max needed:
|s| stays far below f32 exp overflow),  vw = x @ u_h.

Sharding: 64 (n, h) units over 8 cores -> core c handles n = c//2 and 8
heads h0 = 8*(c%2). Per core everything stays on-chip: q/k projections
(bf16 matmuls into f32 PSUM), per-head 2048x2048 causal scores in
transposed orientation (m on partitions), exp on ScalarE (scale=1/8
folded in, bf16 out), and num/den via PE matmuls against
[vw_hi | vw_lo | 1] bf16 columns (hi/lo split keeps the vw path at
f32-class accuracy; vw itself is computed on-chip from hi/lo-split x).
The host divides num/den and sums (tiny, well-conditioned).
"""

import os

import ml_dtypes
import numpy as np

import concourse.bacc as bacc
import concourse.mybir as mybir
import concourse.tile as tile
from concourse.bass_utils import run_bass_kernel_spmd
from concourse.masks import make_identity

ml_bf16 = ml_dtypes.bfloat16

L = 2048
N = 4
E = 1024
H = 16
D = 64
HPC = 8  # heads per core
NCORES = 8
KS = E // 128  # 8 contraction subtiles
LB = 512  # l-block (matmul free dim)
NLB = L // LB  # 4
MB = 128  # m-block (scores partition dim)
NMO = L // MB  # 16
G = 3  # score blocks per PSUM tile / ACT instruction

F32 = mybir.dt.float32
BF16 = mybir.dt.bfloat16

_nc_cache = None


def _emit_main_head(nc, h, qt_sb, kt_sb, mask_sb, redw, spsum, rpsum, epool,
                    ndpool, numden):
    """Scores + exp + mask + num/den reduction for one local head.

    Diagonal blocks are width-trimmed (o=2 -> l in [256,512), o=3 -> l in
    [384,512)); reduction matmuls trail the score matmuls by RED_DELAY
    groups so the PE never stalls waiting for ScalarE's exp."""
    jg = h // 2
    po = (h % 2) * 64
    RED_DELAY = 2
    for lb in range(NLB):
        # entries: (mo, l_off, width, diag_o or None)
        entries = [(mo, 0, LB, None) for mo in range(4 * lb)]
        for o, (off, w) in enumerate(((0, 512), (0, 512), (256, 256), (384, 128))):
            entries.append((4 * lb + o, off, w, o))
        # groups: full blocks in triplets, the 4 diagonal blocks together
        full = entries[: 4 * lb]
        groups = [full[i : i + G] for i in range(0, len(full), G)]
        groups = [g for g in groups if g] + [entries[4 * lb :]]
        n_entries = len(entries)

        psr = rpsum.tile([3, LB], F32, tag="r", name="psr")
        pending = []

        def emit_red(grp_info, first_idx_of_lb=0):
            et, grp, offs = grp_info
            for (mo, off, w, o), poff in zip(grp, offs):
                ei = next(i for i, e in enumerate(entries) if e[0] == mo)
                nc.tensor.matmul(
                    psr[:, off : off + w],
                    redw[:, mo, h, :],
                    et[:, poff : poff + w],
                    start=(ei == 0),
                    stop=(ei == n_entries - 1),
                )

        for gi, grp in enumerate(groups):
            offs = []
            tot = 0
            for _, _, w, _ in grp:
                offs.append(tot)
                tot += w
            ps = spsum.tile([128, G * LB], F32, tag="s", name="ps_scores")
            for (mo, off, w, o), poff in zip(grp, offs):
                nc.tensor.matmul(
                    ps[:, poff : poff + w],
                    kt_sb[po : po + 64, jg, mo * MB : (mo + 1) * MB],
                    qt_sb[po : po + 64, jg, lb * LB + off : lb * LB + off + w],
                    start=True,
                    stop=True,
                    tile_position=(po, 0),
                )
            et = epool.tile([128, G * LB], BF16, tag="e", name="et")
            nc.scalar.activation(
                et[:, :tot], ps[:, :tot],
                mybir.ActivationFunctionType.Exp,
                scale=float(D) ** -0.5,
            )
            for (mo, off, w, o), poff in zip(grp, offs):
                if o is not None:
                    nc.vector.tensor_mul(
                        et[:, poff : poff + w],
                        et[:, poff : poff + w],
                        mask_sb[:, o, off : off + w],
                    )
            pending.append((et, grp, offs))
            if len(pending) > RED_DELAY:
                emit_red(pending.pop(0))
        for grp_info in pending:
            emit_red(grp_info)
        ndt = ndpool.tile([3, LB], F32, tag="nd", name="ndt")
        nc.vector.tensor_copy(ndt[:], psr[:])
        nc.sync.dma_start(numden[:, h, lb, :], ndt[:])


def _build():
    nc = bacc.Bacc(None, target_bir_lowering=False, name="mhsa_sum")

    xt = nc.dram_tensor("xt", [E, L], BF16, kind="ExternalInput")
    xtl = nc.dram_tensor("xtl", [E, L], BF16, kind="ExternalInput")
    wqt = nc.dram_tensor("wqt", [E, HPC * D], BF16, kind="ExternalInput")
    wkt = nc.dram_tensor("wkt", [E, HPC * D], BF16, kind="ExternalInput")
    u2 = nc.dram_tensor("u2", [E, 2, HPC], BF16, kind="ExternalInput")  # hi, lo
    mask = nc.dram_tensor("mask", [128, 4, LB], BF16, kind="ExternalInput")
    numden = nc.dram_tensor("numden", [3, HPC, NLB, LB], F32, kind="ExternalOutput")

    xt3 = xt.rearrange("(ko p) l -> p ko l", p=128)
    xtl3 = xtl.rearrange("(ko p) l -> p ko l", p=128)
    wqt3 = wqt.rearrange("(ko p) j -> p ko j", p=128)
    wkt3 = wkt.rearrange("(ko p) j -> p ko j", p=128)
    u23 = u2.rearrange("(ko p) t h -> p ko t h", p=128)

    with tile.TileContext(nc) as tc:
        with (
            tc.tile_pool(name="const", bufs=1) as const,
            tc.tile_pool(name="epool", bufs=6) as epool,
            tc.tile_pool(name="ndpool", bufs=2) as ndpool,
            tc.tile_pool(name="spsum", bufs=2, space="PSUM") as spsum,
            tc.tile_pool(name="rpsum", bufs=2, space="PSUM") as rpsum,
        ):
            wq_sb = const.tile([128, KS, HPC * D], BF16)
            wk_sb = const.tile([128, KS, HPC * D], BF16)
            u_sb = const.tile([128, KS, 2, HPC], BF16)
            mask_sb = const.tile([128, 4, LB], BF16)
            xt_sb = const.tile([128, KS, L], BF16)
            xtl_sb = const.tile([128, KS, L], BF16)
            qt_sb = const.tile([128, 4, L], BF16)
            kt_sb = const.tile([128, 4, L], BF16)
            redw = const.tile([128, NMO, HPC, 3], BF16)
            ident8 = const.tile([8, 8], F32)
            vwsb = const.tile([8, L], F32)

            nc.sync.dma_start(u_sb[:], u23[:])
            nc.sync.dma_start(wq_sb[:], wqt3[:])
            nc.sync.dma_start(wk_sb[:], wkt3[:])
            nc.sync.dma_start(mask_sb[:], mask[:])
            for ks in range(KS):
                nc.sync.dma_start(xt_sb[:, ks, :], xt3[:, ks, :])
            for ks in range(KS):
                nc.sync.dma_start(xtl_sb[:, ks, :], xtl3[:, ks, :])

            make_identity(nc, ident8[:])
            nc.vector.memset(redw[:, :, :, 2], 1.0)

            # vw[h, m] = sum_e x[m, e] u[e, h], natural orientation, three
            # bf16 hi/lo cross terms accumulated in f32 PSUM.
            for lbs in ((0, 1, 2), (3,)):
                psv = spsum.tile([8, len(lbs) * LB], F32, tag="s", name="psv")
                for li, lb in enumerate(lbs):
                    for ti, (xs, ut) in enumerate(
                        ((xt_sb, 0), (xt_sb, 1), (xtl_sb, 0))
                    ):
                        for ks in range(KS):
                            nc.tensor.matmul(
                                psv[:, li * LB : (li + 1) * LB],
                                u_sb[:, ks, ut, :],
                                xs[:, ks, lb * LB : (lb + 1) * LB],
                                start=(ti == 0 and ks == 0),
                                stop=(ti == 2 and ks == KS - 1),
                            )
                nc.vector.tensor_copy(
                    vwsb[:, lbs[0] * LB : (lbs[-1] + 1) * LB], psv[:]
                )

            # transpose vw to [m, h] per 128-m-block, then split-store
            # hi/lo bf16 into the reduction-weight columns.
            for mo in range(NMO):
                psT = rpsum.tile([128, 8], F32, tag="r", name="psT")
                nc.tensor.transpose(
                    psT[:], vwsb[:, mo * MB : (mo + 1) * MB], ident8[:]
                )
                nc.vector.tensor_copy(redw[:, mo, :, 0], psT[:])
                nc.vector.tensor_sub(redw[:, mo, :, 1], psT[:], redw[:, mo, :, 0])

            # q/k projections interleaved with per-head-pair main loops so
            # the PE never drains while ACT catches up.
            for jg in range(4):
                for w_sb, out_sb in ((wq_sb, qt_sb), (wk_sb, kt_sb)):
                    for lbs in ((0, 1, 2), (3,)):
                        ps = spsum.tile([128, G * LB], F32, tag="s", name="ps_proj")
                        for li, lb in enumerate(lbs):
                            for ks in range(KS):
                                nc.tensor.matmul(
                                    ps[:, li * LB : (li + 1) * LB],
                                    w_sb[:, ks, jg * 128 : (jg + 1) * 128],
                                    xt_sb[:, ks, lb * LB : (lb + 1) * LB],
                                    start=(ks == 0),
                                    stop=(ks == KS - 1),
                                )
                        nc.vector.tensor_copy(
                            out_sb[:, jg, lbs[0] * LB : (lbs[-1] + 1) * LB],
                            ps[:, : len(lbs) * LB],
                        )
                for h in (2 * jg, 2 * jg + 1):
                    _emit_main_head(nc, h, qt_sb, kt_sb, mask_sb, redw,
                                    spsum, rpsum, epool, ndpool, numden)

    nc.compile()
    return nc


def _split_bf16(a):
    hi = a.astype(np.float32).astype(ml_bf16)
    lo = (a.astype(np.float32) - hi.astype(np.float32)).astype(ml_bf16)
    return hi, lo


def _prep_core_inputs(x, wqkv, wout):
    """Host-side sharding + weight folding (data marshaling only)."""
    x = np.ascontiguousarray(np.asarray(x, dtype=np.float32))
    wqkv = np.asarray(wqkv, dtype=np.float32)
    wout = np.asarray(wout, dtype=np.float32)

    wsum = wout.astype(np.float64).sum(axis=0)  # [E]

    # causal mask for transposed-score diagonal blocks:
    # valid when f >= 128*o + p
    p = np.arange(128)[:, None, None]
    o = np.arange(4)[None, :, None]
    f = np.arange(LB)[None, None, :]
    mask_np = (f >= 128 * o + p).astype(ml_bf16)

    in_maps = []
    for c in range(NCORES):
        n = c // 2
        h0 = HPC * (c % 2)
        xt_f32 = np.ascontiguousarray(x[:, n, :].T)  # [E, L]
        xt_hi, xt_lo = _split_bf16(xt_f32)
        wq = wqkv[h0 * D : (h0 + HPC) * D, :]  # [512, E]
        wk = wqkv[E + h0 * D : E + (h0 + HPC) * D, :]
        wqt = np.ascontiguousarray(wq.T).astype(ml_bf16)
        wkt = np.ascontiguousarray(wk.T).astype(ml_bf16)
        u = np.empty((E, HPC), dtype=np.float32)
        for i in range(HPC):
            h = h0 + i
            wv_h = wqkv[2 * E + h * D : 2 * E + (h + 1) * D, :].astype(np.float64)
            u[:, i] = (wv_h.T @ wsum[h * D : (h + 1) * D]).astype(np.float32)
        u_hi, u_lo = _split_bf16(u)
        u2 = np.stack([u_hi, u_lo], axis=1)  # [E, 2, HPC]
        in_maps.append(
            {
                "xt": xt_hi,
                "xtl": xt_lo,
                "wqt": wqt,
                "wkt": wkt,
                "u2": np.ascontiguousarray(u2),
                "mask": mask_np,
            }
        )
    return in_maps


def kernel(x, wqkv, wout):
    global _nc_cache
    if _nc_cache is None:
        _nc_cache = _build()
    nc = _nc_cache

    in_maps = _prep_core_inputs(x, wqkv, wout)
    trace = os.environ.get("KERNEL_TRACE") == "1"
    res = run_bass_kernel_spmd(
        nc, in_maps, core_ids=list(range(NCORES)), trace=trace
    )
    if trace:
        kernel.last_result = res

    total = 0.0
    for c in range(NCORES):
        nd = res.results[c]["numden"].astype(np.float64)
        total += ((nd[0] + nd[1]) / nd[2]).sum()
    return np.array(total, dtype=np.float32)
